# revision 1
# baseline (speedup 1.0000x reference)
"""GRUCell + LayerNorm readout fused Bass kernel for Trainium2 (8 NeuronCores).

Problem: B=8192, D=H=O=1024 fp32.
    r = sigmoid(x@Wir + bir + h@Whr)
    z = sigmoid(x@Wiz + biz + h@Whz)
    n = tanh(x@Win + bin_ + r*(h@Whn + bhn))
    new_h = (1-z)*n + z*h
    out = (LayerNorm(new_h)*ln_scale + ln_bias) @ Wout + bout

Strategy:
  - Data-parallel over batch: core c gets rows [c*1024, (c+1)*1024); weights
    replicated. No collectives.
  - Everything computed in the transposed domain: activations live as
    [feature, batch] so matmuls take the weights in natural [k, h] layout as
    the stationary operand and xT/hT as the moving operand, and the per-h gate
    biases become per-partition activation biases. Host passes xT/hT and
    transposes the outputs back.
  - float32r matmuls: 4x faster than fp32 on the PE at ~1.5e-4 rel error.
  - Matmuls are emitted k-major with both batch chunks interleaved so the PE
    can start as soon as the first input slices land (DMA-matched ramp); the
    8 gate accumulators occupy all 8 PSUM banks.
  - LayerNorm reduces over h (= partition dim): per-tile partials accumulate
    elementwise on GpSimd, one partition_all_reduce per stat at the end.
    The normalize-then-matmul is algebraically folded:
        LN(new_h) @ (ln_scale*Wout) + (ln_bias@Wout + bout)
      = rstd[b]*( new_h@WoutF - mu[b]*colsum[o] ) + boutF[o]
    with WoutF = ln_scale[:,None]*Wout (host), colsum = ln_scale@Wout (host),
    boutF = bout + ln_bias@Wout (host); the mu*colsum term is a K=1 rank-1
    matmul into the same PSUM accumulator.
"""

import sys
from contextlib import ExitStack

sys.path.insert(0, "/opt/trn_rl_repo")

import numpy as np

import concourse.bacc as bacc
import concourse.mybir as mybir
import concourse.tile as tile
from concourse import bass_isa, bass_utils

B, D, H, O = 8192, 1024, 1024, 1024
NCORES = 8
BL = B // NCORES          # batch rows per core
P = 128                   # partitions
KT = D // P               # contraction tiles (8)
HT = H // P               # h output-partition tiles (8)
OT = O // P               # o output-partition tiles (8)
NB = 2                    # batch chunks per core (free dim 512)
NF = BL // NB             # free dim per chunk (512)
LN_EPS = 1e-6

F32 = mybir.dt.float32
F32R = mybir.dt.float32r

_COMPILED = None  # compiled Bacc module cache across calls
TRACE = False     # set by test harness to capture an NTFF profile
LAST_RES = None   # BassKernelResults of the last run (for the test harness)

XGATES = ("ir", "iz", "in")
HGATES = ("hr", "hz", "hn")


def _build():
    nc = bacc.Bacc("TRN2", target_bir_lowering=False, debug=False,
                   num_devices=NCORES)

    def din(name, shape, dt=F32R):
        return nc.dram_tensor(name, shape, dt, kind="ExternalInput").ap()

    def dout(name, shape, dt=F32):
        return nc.dram_tensor(name, shape, dt, kind="ExternalOutput").ap()

    xT_d = din("xT", [D, BL])
    hT_d = din("hT", [H, BL])
    w_d = {g: din(f"W{g}", [D, H]) for g in XGATES + HGATES}
    woutF_d = din("woutF", [H, O])
    bir_d = din("bir", [H], F32)
    biz_d = din("biz", [H], F32)
    bin_d = din("bin", [H], F32)
    bhn_d = din("bhn", [H], F32)
    boutF_d = din("boutF", [O], F32)
    colsum_d = din("colsum", [1, O])
    ones_col_d = din("ones_col", [P, 1])
    ones_row_d = din("ones_row", [1, P])

    nhT_d = dout("nhT", [H, BL])
    outT_d = dout("outT", [O, BL])

    with tile.TileContext(nc) as tc, ExitStack() as ctx:
        singles = ctx.enter_context(tc.tile_pool(name="singles", bufs=1))
        wpool = ctx.enter_context(tc.tile_pool(name="wpool", bufs=2))
        gates = ctx.enter_context(tc.tile_pool(name="gates", bufs=1))
        rows = ctx.enter_context(tc.tile_pool(name="rows", bufs=1))
        ps = ctx.enter_context(tc.tile_pool(name="ps", bufs=1, space="PSUM"))

        # ---- resident inputs, DMA-ordered to feed the PE ramp ---------------
        def kslice_tile(prefix, k):
            return singles.tile([P, BL], F32R, tag=f"{prefix}{k}",
                                name=f"{prefix}{k}")

        def load_w(g, ht):
            t = wpool.tile([P, KT, P], F32R, tag=f"w{g}", name=f"w{g}_{ht}")
            nc.sync.dma_start(t[:], w_d[g][:, ht * P:(ht + 1) * P].rearrange(
                "(t p) h -> p t h", p=P))
            return t

        xT_sb, hT_sb = [], []
        for k in range(KT):
            xT_sb.append(kslice_tile("xk", k))
            hT_sb.append(kslice_tile("hk", k))

        # x slice 0, then the x-side weights for ht=0, then the rest of x,
        # then the h-side weights for ht=0, then h.
        nc.sync.dma_start(xT_sb[0][:], xT_d[0:P, :])
        w0 = {g: load_w(g, 0) for g in XGATES}
        for k in range(1, KT):
            nc.sync.dma_start(xT_sb[k][:], xT_d[k * P:(k + 1) * P, :])
        w0["hr"] = load_w("hr", 0)
        nc.sync.dma_start(hT_sb[0][:], hT_d[0:P, :])
        w0["hz"] = load_w("hz", 0)
        nc.sync.dma_start(hT_sb[1][:], hT_d[P:2 * P, :])
        w0["hn"] = load_w("hn", 0)
        for k in range(2, KT):
            nc.sync.dma_start(hT_sb[k][:], hT_d[k * P:(k + 1) * P, :])

        def load_vec(ap_d, n, tag):
            t = singles.tile([P, n // P], F32, tag=tag, name=tag)
            nc.sync.dma_start(t[:], ap_d.rearrange("(t p) -> p t", p=P))
            return t

        bir_sb = load_vec(bir_d, H, "bir_sb")
        biz_sb = load_vec(biz_d, H, "biz_sb")
        bin_sb = load_vec(bin_d, H, "bin_sb")
        bhn_sb = load_vec(bhn_d, H, "bhn_sb")
        boutF_sb = load_vec(boutF_d, O, "boutF_sb")
        colsum_sb = singles.tile([1, O], F32R)
        nc.sync.dma_start(colsum_sb[:], colsum_d)
        ones_col = singles.tile([P, 1], F32R)
        nc.sync.dma_start(ones_col[:], ones_col_d)
        ones_row = singles.tile([1, P], F32R)
        nc.sync.dma_start(ones_row[:], ones_row_d)
        eps_sb = singles.tile([1, 1], F32)
        nc.vector.memset(eps_sb[:], LN_EPS)

        new_hT_sb = [singles.tile([P, BL], F32R, tag=f"nh{ht}",
                                  name=f"nh{ht}") for ht in range(HT)]
        s_acc = [singles.tile([P, NF], F32R, tag=f"s_acc{bc}",
                              name=f"s_acc{bc}") for bc in range(NB)]
        q_acc = [singles.tile([P, NF], F32R, tag=f"q_acc{bc}",
                              name=f"q_acc{bc}") for bc in range(NB)]

        # ---- phase 1: gates + new_h -----------------------------------------
        woutF_sb = singles.tile([P, KT, O], F32R)

        for ht in range(HT):
            hs = slice(ht * P, (ht + 1) * P)
            w_sb = w0 if ht == 0 else {g: load_w(g, ht)
                                       for g in XGATES + HGATES}
            if ht == 2:
                # readout weights: resident; loaded after the ramp-critical
                # input/gate-weight prefetches are in flight
                nc.sync.dma_start(woutF_sb[:], woutF_d.rearrange(
                    "(t p) o -> p t o", p=P))

            pr = [ps.tile([P, NF], F32, tag=f"r{bc}", name=f"pr{bc}_{ht}")
                  for bc in range(NB)]
            pz = [ps.tile([P, NF], F32, tag=f"z{bc}", name=f"pz{bc}_{ht}")
                  for bc in range(NB)]
            pgi = [ps.tile([P, NF], F32, tag=f"gi{bc}", name=f"pgi{bc}_{ht}")
                   for bc in range(NB)]
            pgh = [ps.tile([P, NF], F32, tag=f"gh{bc}", name=f"pgh{bc}_{ht}")
                   for bc in range(NB)]

            bsl = [slice(bc * NF, (bc + 1) * NF) for bc in range(NB)]

            # k-major, both batch chunks interleaved: x side then h side.
            for k in range(KT):
                for bc in range(NB):
                    xs = xT_sb[k][:, bsl[bc]]
                    nc.tensor.matmul(pr[bc][:], w_sb["ir"][:, k, :], xs,
                                     start=(k == 0), stop=False)
                    nc.tensor.matmul(pz[bc][:], w_sb["iz"][:, k, :], xs,
                                     start=(k == 0), stop=False)
                    nc.tensor.matmul(pgi[bc][:], w_sb["in"][:, k, :], xs,
                                     start=(k == 0), stop=(k == KT - 1))
            for k in range(KT):
                for bc in range(NB):
                    hss = hT_sb[k][:, bsl[bc]]
                    nc.tensor.matmul(pr[bc][:], w_sb["hr"][:, k, :], hss,
                                     start=False, stop=(k == KT - 1))
                    nc.tensor.matmul(pz[bc][:], w_sb["hz"][:, k, :], hss,
                                     start=False, stop=(k == KT - 1))
                    nc.tensor.matmul(pgh[bc][:], w_sb["hn"][:, k, :], hss,
                                     start=(k == 0), stop=(k == KT - 1))

            for bc in range(NB):
                bs = bsl[bc]
                r_sb = gates.tile([P, NF], F32, tag="r_act")
                nc.scalar.activation(r_sb[:], pr[bc][:],
                                     mybir.ActivationFunctionType.Sigmoid,
                                     bias=bir_sb[:, ht:ht + 1])
                z_sb = gates.tile([P, NF], F32, tag="z_act")
                nc.scalar.activation(z_sb[:], pz[bc][:],
                                     mybir.ActivationFunctionType.Sigmoid,
                                     bias=biz_sb[:, ht:ht + 1])

                t_sb = gates.tile([P, NF], F32, tag="t")
                nc.vector.tensor_scalar(t_sb[:], pgh[bc][:],
                                        bhn_sb[:, ht:ht + 1],
                                        None, mybir.AluOpType.add)
                nc.vector.tensor_mul(t_sb[:], t_sb[:], r_sb[:])
                nc.vector.tensor_add(t_sb[:], t_sb[:], pgi[bc][:])
                n_sb = gates.tile([P, NF], F32, tag="r_act", name="n_sb")
                nc.scalar.activation(n_sb[:], t_sb[:],
                                     mybir.ActivationFunctionType.Tanh,
                                     bias=bin_sb[:, ht:ht + 1])

                u_sb = gates.tile([P, NF], F32, tag="u")
                nc.vector.tensor_tensor(u_sb[:], hT_sb[ht][:, bs].bitcast(F32),
                                        n_sb[:], mybir.AluOpType.subtract)
                nc.vector.tensor_mul(u_sb[:], z_sb[:], u_sb[:])
                nh = new_hT_sb[ht][:, bs]
                nc.vector.tensor_add(nh, n_sb[:], u_sb[:])
                nhf = nh.bitcast(F32)

                # LN stat partials: elementwise accumulate over h-tiles (DVE),
                # cross-partition reduce later via a ones-column matmul.
                sq_sb = gates.tile([P, NF], F32R, tag="t", name="sq_sb")
                if ht == 0:
                    nc.vector.tensor_copy(s_acc[bc][:], nhf)
                    nc.scalar.activation(q_acc[bc][:], nhf,
                                         mybir.ActivationFunctionType.Square)
                else:
                    nc.vector.tensor_tensor(s_acc[bc][:],
                                            s_acc[bc][:].bitcast(F32), nhf,
                                            mybir.AluOpType.add)
                    nc.scalar.activation(sq_sb[:], nhf,
                                         mybir.ActivationFunctionType.Square)
                    nc.vector.tensor_tensor(q_acc[bc][:],
                                            q_acc[bc][:].bitcast(F32),
                                            sq_sb[:].bitcast(F32),
                                            mybir.AluOpType.add)

                # stores go through GpSimd's DMA queue so they never
                # head-of-line-block weight loads on the Sync queue
                nc.gpsimd.dma_start(nhT_d[hs, bs], nhf)

        # ---- phase 2: LN scale factors + readout ----------------------------
        # bc=0 groups run first so the bc=1 stats chain hides under them; the
        # stats reduce-matmuls, broadcast matmuls, rank-1s, and epilogues are
        # all software-pipelined into the main matmul stream so the in-order
        # PE never stalls on the stats chain or cools down (HAM).
        red_tags = ("gi0", "gi1", "gh0", "gh1")
        nmu_row = {}
        rstd_row = {}
        rstd_bc = {}

        def emit_stats(bc):
            psum_s = ps.tile([1, NF], F32, tag=red_tags[2 * bc],
                             name=f"psum_s{bc}")
            nc.tensor.matmul(psum_s[:], ones_col[:], s_acc[bc][:],
                             start=True, stop=True)
            psum_q = ps.tile([1, NF], F32, tag=red_tags[2 * bc + 1],
                             name=f"psum_q{bc}")
            nc.tensor.matmul(psum_q[:], ones_col[:], q_acc[bc][:],
                             start=True, stop=True)

            nmu = rows.tile([1, NF], F32R, tag=f"nmu{bc}", name=f"nmu{bc}")
            nc.vector.tensor_scalar_mul(nmu[:], psum_s[:], -1.0 / H)
            nmu_row[bc] = nmu

            mu2 = gates.tile([1, NF], F32, tag="t", name=f"mu2_{bc}")
            nc.vector.tensor_mul(mu2[:], nmu[:].bitcast(F32), nmu[:].bitcast(F32))
            var = gates.tile([1, NF], F32, tag="u", name=f"var_{bc}")
            nc.vector.tensor_scalar_mul(var[:], psum_q[:], 1.0 / H)
            nc.vector.tensor_tensor(var[:], var[:], mu2[:],
                                    mybir.AluOpType.subtract)
            nc.scalar.activation(var[:], var[:],
                                 mybir.ActivationFunctionType.Sqrt,
                                 bias=eps_sb[:])
            rrow = gates.tile([1, NF], F32R, tag=("z_act", "r_act")[bc],
                              name=f"rstd{bc}")
            with nc.allow_low_precision(reason="f32r is fp32-width"):
                nc.vector.reciprocal(rrow[:], var[:])
            rstd_row[bc] = rrow

        po_tags = ("r0", "z0", "r1", "z1", "gh0", "gh1")
        PIPE = 5
        groups = [(ot, bc) for bc in range(NB) for ot in range(OT)]
        pending = {}

        def finalize(i):
            ot, bc = groups[i]
            po = pending.pop(i)
            os_ = slice(ot * P, (ot + 1) * P)
            bs = slice(bc * NF, (bc + 1) * NF)
            # -= mu[b] * colsum[o]  (rank-1, K=1)
            nc.tensor.matmul(po[:], colsum_sb[0:1, os_], nmu_row[bc][:],
                             start=False, stop=True)
            o_sb = gates.tile([P, NF], F32, tag=("t", "u", "z_act")[i % 3],
                              name=f"o_{ot}_{bc}")
            nc.vector.tensor_mul(o_sb[:], po[:], rstd_bc[bc][:])
            nc.vector.tensor_scalar(o_sb[:], o_sb[:],
                                    boutF_sb[:, ot:ot + 1], None,
                                    mybir.AluOpType.add)
            nc.gpsimd.dma_start(outT_d[os_, bs], o_sb[:])

        def emit_pb(bc):
            pb = ps.tile([P, NF], F32, tag=red_tags[bc], name=f"pb{bc}")
            nc.tensor.matmul(pb[:], ones_row[:], rstd_row[bc][:],
                             start=True, stop=True)
            rb = rows.tile([P, NF], F32, tag=f"rstd_bc{bc}",
                           name=f"rstd_bc{bc}")
            nc.vector.tensor_copy(rb[:], pb[:])
            rstd_bc[bc] = rb

        done = 0
        for i, (ot, bc) in enumerate(groups):
            bs = slice(bc * NF, (bc + 1) * NF)
            po = ps.tile([P, NF], F32, tag=po_tags[i % len(po_tags)],
                         name=f"po_{ot}_{bc}")
            for k in range(HT):
                nc.tensor.matmul(po[:], woutF_sb[:, k, ot * P:(ot + 1) * P],
                                 new_hT_sb[k][:, bs],
                                 start=(k == 0), stop=False)
            pending[i] = po
            if i == 0:
                emit_stats(0)
            elif i == 1:
                emit_stats(1)
            elif i == 3:
                emit_pb(0)
            elif i == 7:
                emit_pb(1)
            if i >= PIPE:
                finalize(done)
                done += 1
            if i >= 8 and done <= i - 1:
                # drain the pipeline early so the tail is short
                finalize(done)
                done += 1
        while done < len(groups):
            finalize(done)
            done += 1

    nc.compile()
    return nc


def kernel(x, h, Wir, bir, Wiz, biz, Win, bin_, Whr, Whz, Whn, bhn,
           ln_scale, ln_bias, Wout, bout):
    global _COMPILED, LAST_RES
    if _COMPILED is None:
        _COMPILED = _build()
    nc = _COMPILED

    x = np.asarray(x, np.float32)
    h = np.asarray(h, np.float32)
    xT = np.ascontiguousarray(x.T)
    hT = np.ascontiguousarray(h.T)
    Wout = np.asarray(Wout, np.float32)
    ln_scale = np.asarray(ln_scale, np.float32)
    ln_bias = np.asarray(ln_bias, np.float32)
    woutF = np.ascontiguousarray(ln_scale[:, None] * Wout)
    boutF = np.asarray(bout, np.float32) + ln_bias @ Wout
    colsum = (ln_scale @ Wout).reshape(1, O)

    common = {
        "Wir": np.asarray(Wir, np.float32), "Wiz": np.asarray(Wiz, np.float32),
        "Win": np.asarray(Win, np.float32), "Whr": np.asarray(Whr, np.float32),
        "Whz": np.asarray(Whz, np.float32), "Whn": np.asarray(Whn, np.float32),
        "woutF": woutF,
        "bir": np.asarray(bir, np.float32), "biz": np.asarray(biz, np.float32),
        "bin": np.asarray(bin_, np.float32), "bhn": np.asarray(bhn, np.float32),
        "boutF": boutF.astype(np.float32), "colsum": colsum.astype(np.float32),
        "ones_col": np.ones((P, 1), np.float32),
        "ones_row": np.ones((1, P), np.float32),
    }
    in_maps = []
    for c in range(NCORES):
        bsl = slice(c * BL, (c + 1) * BL)
        in_maps.append({
            **common,
            "xT": np.ascontiguousarray(xT[:, bsl]),
            "hT": np.ascontiguousarray(hT[:, bsl]),
        })

    res = bass_utils.run_bass_kernel_spmd(nc, in_maps,
                                          core_ids=list(range(NCORES)),
                                          trace=TRACE)
    LAST_RES = res
    new_hT = np.concatenate([res.results[c]["nhT"] for c in range(NCORES)],
                            axis=1)
    outT = np.concatenate([res.results[c]["outT"] for c in range(NCORES)],
                          axis=1)
    new_h = np.ascontiguousarray(new_hT.T)
    out = np.ascontiguousarray(outT.T)
    return new_h, out



# revision 10
# speedup vs baseline: 1.2058x; 1.2058x over previous
"""GRUCell + LayerNorm readout fused Bass kernel for Trainium2 (8 NeuronCores).

Problem: B=8192, D=H=O=1024 fp32.
    r = sigmoid(x@Wir + bir + h@Whr)
    z = sigmoid(x@Wiz + biz + h@Whz)
    n = tanh(x@Win + bin_ + r*(h@Whn + bhn))
    new_h = (1-z)*n + z*h
    out = (LayerNorm(new_h)*ln_scale + ln_bias) @ Wout + bout

Strategy (v2):
  - Data-parallel over batch: core c gets rows [c*1024, (c+1)*1024); weights
    replicated. No collectives.
  - Transposed domain: activations live as [feature, batch]; per-h gate
    biases become per-partition activation biases.
  - All big matmuls in bf16 (host pre-casts + pre-swizzles weights/inputs to
    the exact SBUF layout, so every DMA is contiguous 2KB-per-partition).
    Measured end-to-end bf16 error ~3e-3 << 2e-2 gate. Stats/broadcast/rank-1
    matmuls stay f32r (exact fp32 width).
  - PSUM A/B alternation: each (ht, bc) group of 48 matmuls uses 4 banks
    (r,z,gi,gh) of one parity; its ~3.3us epilogue chain drains under the
    next group's 10.2us matmul stream, so the PE never waits on PSUM reuse.
  - PE warmup: ~24 dummy matmuls at t=0 cover the input-DMA ramp and lift
    the HAM clock gate (1.2->2.4GHz) before real work arrives.
  - DMA queues: sync=weight stream + outT stores, scalar=x/h/bias inputs,
    gpsimd=woutF chunks + nhT stores. Weight stream is 1.5MB/ht vs 20.4us/ht
    of PE work, so the queue never back-pressures the PE (the old f32
    strided weight stream did, costing an 8.4us stall + a HAM re-throttle).
  - LayerNorm folded into the readout:
        LN(new_h) @ (ln_scale*Wout) + (ln_bias@Wout + bout)
      = rstd[b]*( new_h@WoutF - mu[b]*colsum[o] ) + boutF[o]
    with the mu*colsum term as a K=1 rank-1 matmul into the same PSUM
    accumulator; LN stats accumulate elementwise over h-tiles then reduce
    across partitions via a ones-column matmul. Phase-2 finalize is eager:
    each readout group's rank-1 + 2 DVE ops + store fire as soon as its
    8-matmul accumulation stops, so the tail is one group's epilogue.
"""

import sys
from contextlib import ExitStack

sys.path.insert(0, "/opt/trn_rl_repo")

import numpy as np
import ml_dtypes

import concourse.bacc as bacc
import concourse.mybir as mybir
import concourse.tile as tile
from concourse import bass_utils

B, D, H, O = 8192, 1024, 1024, 1024
NCORES = 8
BL = B // NCORES          # batch rows per core
P = 128                   # partitions
KT = D // P               # contraction tiles (8)
HT = H // P               # h output-partition tiles (8)
OT = O // P               # o output-partition tiles (8)
NB = 2                    # batch chunks per core
NF = BL // NB             # free dim per chunk (512)
LN_EPS = 1e-6
NWARM = 24                # PE warmup matmuls

F32 = mybir.dt.float32
F32R = mybir.dt.float32r
BF16 = mybir.dt.bfloat16
NPBF16 = ml_dtypes.bfloat16

_COMPILED = None
TRACE = False
LAST_RES = None

GATES = ("ir", "iz", "in", "hr", "hz", "hn")


def _build():
    nc = bacc.Bacc("TRN2", target_bir_lowering=False, debug=False,
                   num_devices=NCORES)

    xT_d = nc.dram_tensor("xT", [KT, P, BL], BF16, kind="ExternalInput").ap()
    hT_d = nc.dram_tensor("hT", [KT, P, BL], BF16, kind="ExternalInput").ap()
    wall_d = nc.dram_tensor("wall", [HT, len(GATES), P, KT, P], BF16,
                            kind="ExternalInput").ap()
    wout_d = nc.dram_tensor("woutF", [OT, P, KT, P], BF16,
                            kind="ExternalInput").ap()
    # [P, 41]: bir | biz | bin | bhn | boutF (8 cols each, col t = tile t),
    # col 40 = ones (stats-reduce stationary)
    bvec_d = nc.dram_tensor("bvec", [P, 41], F32R, kind="ExternalInput").ap()
    # [1, P+O]: ones_row | colsum
    rowv_d = nc.dram_tensor("rowv", [1, P + O], F32R,
                            kind="ExternalInput").ap()

    nhT_d = nc.dram_tensor("nhT", [HT, P, BL], BF16,
                           kind="ExternalOutput").ap()
    outT_d = nc.dram_tensor("outT", [OT, P, BL], BF16,
                            kind="ExternalOutput").ap()

    with tile.TileContext(nc) as tc, ExitStack() as ctx:
        singles = ctx.enter_context(tc.tile_pool(name="singles", bufs=1))
        wpool = ctx.enter_context(tc.tile_pool(name="wpool", bufs=2))
        gates = ctx.enter_context(tc.tile_pool(name="gates", bufs=1))
        rows = ctx.enter_context(tc.tile_pool(name="rows", bufs=1))
        ps = ctx.enter_context(tc.tile_pool(name="ps", bufs=1, space="PSUM"))

        TAGS = [["a0", "b0", "c0", "d0"], ["a1", "b1", "c1", "d1"]]

        # ---- PE warmup: dummy matmuls cover the DMA ramp, lift HAM -------
        wm = singles.tile([P, 2 * P], BF16, tag="wm", name="wm")
        nc.vector.memset(wm[:], 0.0)
        for i in range(NWARM):
            pw = ps.tile([P, P], F32, tag="a1", name=f"warm{i}")
            nc.tensor.matmul(pw[:], wm[:, 0:P], wm[:, P:2 * P],
                             start=True, stop=True)

        # ---- resident inputs --------------------------------------------
        x_sb = singles.tile([P, KT, BL], BF16, tag="x_sb", name="x_sb")
        h_sb = singles.tile([P, KT, BL], BF16, tag="h_sb", name="h_sb")

        def load_wht(ht):
            d = {}
            for gi, g in enumerate(GATES):
                t = wpool.tile([P, KT, P], BF16, tag=f"w{g}",
                               name=f"w{g}_{ht}")
                nc.sync.dma_start(t[:], wall_d[ht, gi])
                d[g] = t
            return d

        # first weight tile + first x slice race to start the PE
        w_cur = load_wht(0)
        for k in range(KT):
            nc.scalar.dma_start(x_sb[:, k, :], xT_d[k])
            nc.scalar.dma_start(h_sb[:, k, :], hT_d[k])
        w_nxt = load_wht(1)

        bvec = singles.tile([P, 41], F32R, tag="bvec", name="bvec")
        nc.scalar.dma_start(bvec[:], bvec_d)
        rowv = singles.tile([1, P + O], F32R, tag="rowv", name="rowv")
        nc.scalar.dma_start(rowv[:], rowv_d)
        eps_sb = singles.tile([1, 1], F32, tag="eps", name="eps")
        nc.vector.memset(eps_sb[:], LN_EPS)

        def bias_col(v, t):  # v: 0=bir 1=biz 2=bin 3=bhn 4=boutF
            return bvec[:, 8 * v + t:8 * v + t + 1].bitcast(F32)

        ones_col = bvec[:, 40:41]
        ones_row = rowv[:, 0:P]

        def colsum_sl(ot):
            return rowv[:, P + ot * P:P + (ot + 1) * P]

        wout_sb = singles.tile([P, OT, KT, P], BF16, tag="wout",
                               name="wout_sb")

        nh_sb = [singles.tile([P, BL], BF16, tag=f"nh{ht}", name=f"nh{ht}")
                 for ht in range(HT)]
        s_acc = [singles.tile([P, NF], F32R, tag=f"s_acc{bc}",
                              name=f"s_acc{bc}") for bc in range(NB)]
        q_acc = [singles.tile([P, NF], F32R, tag=f"q_acc{bc}",
                              name=f"q_acc{bc}") for bc in range(NB)]

        # ---- phase 1: gates + new_h --------------------------------------
        gidx = 0  # global group counter -> PSUM parity
        for ht in range(HT):
            for bc in range(NB):
                s = gidx % 2
                bs = slice(bc * NF, (bc + 1) * NF)
                pr = ps.tile([P, NF], F32, tag=TAGS[s][0],
                             name=f"pr{ht}_{bc}")
                pz = ps.tile([P, NF], F32, tag=TAGS[s][1],
                             name=f"pz{ht}_{bc}")
                pgi = ps.tile([P, NF], F32, tag=TAGS[s][2],
                              name=f"pgi{ht}_{bc}")
                pgh = ps.tile([P, NF], F32, tag=TAGS[s][3],
                              name=f"pgh{ht}_{bc}")

                for k in range(KT):
                    xs = x_sb[:, k, bs]
                    nc.tensor.matmul(pr[:], w_cur["ir"][:, k, :], xs,
                                     start=(k == 0), stop=False)
                    nc.tensor.matmul(pz[:], w_cur["iz"][:, k, :], xs,
                                     start=(k == 0), stop=False)
                    nc.tensor.matmul(pgi[:], w_cur["in"][:, k, :], xs,
                                     start=(k == 0), stop=(k == KT - 1))
                for k in range(KT):
                    hs = h_sb[:, k, bs]
                    nc.tensor.matmul(pr[:], w_cur["hr"][:, k, :], hs,
                                     start=False, stop=(k == KT - 1))
                    nc.tensor.matmul(pz[:], w_cur["hz"][:, k, :], hs,
                                     start=False, stop=(k == KT - 1))
                    nc.tensor.matmul(pgh[:], w_cur["hn"][:, k, :], hs,
                                     start=(k == 0), stop=(k == KT - 1))

                # epilogue: drains under the next group's matmul stream
                r_sb = gates.tile([P, NF], F32, tag="r_act", name="r_sb")
                nc.scalar.activation(r_sb[:], pr[:],
                                     mybir.ActivationFunctionType.Sigmoid,
                                     bias=bias_col(0, ht))
                z_sb = gates.tile([P, NF], F32, tag="z_act", name="z_sb")
                nc.scalar.activation(z_sb[:], pz[:],
                                     mybir.ActivationFunctionType.Sigmoid,
                                     bias=bias_col(1, ht))

                t_sb = gates.tile([P, NF], F32, tag="t", name="t_sb")
                nc.vector.tensor_scalar(t_sb[:], pgh[:], bias_col(3, ht),
                                        None, mybir.AluOpType.add)
                nc.vector.tensor_mul(t_sb[:], t_sb[:], r_sb[:])
                nc.vector.tensor_add(t_sb[:], t_sb[:], pgi[:])
                n_sb = gates.tile([P, NF], F32, tag="n", name="n_sb")
                nc.scalar.activation(n_sb[:], t_sb[:],
                                     mybir.ActivationFunctionType.Tanh,
                                     bias=bias_col(2, ht))

                u_sb = gates.tile([P, NF], F32, tag="u", name="u_sb")
                nc.vector.tensor_tensor(u_sb[:], h_sb[:, ht, bs], n_sb[:],
                                        mybir.AluOpType.subtract)
                nc.vector.tensor_mul(u_sb[:], z_sb[:], u_sb[:])
                nhv = nh_sb[ht][:, bs]
                nc.vector.tensor_add(nhv, n_sb[:], u_sb[:])

                # LN stat partials (elementwise over h-tiles)
                if ht == 0:
                    nc.vector.tensor_copy(s_acc[bc][:], nhv)
                    nc.scalar.activation(q_acc[bc][:], nhv,
                                         mybir.ActivationFunctionType.Square)
                else:
                    sq_sb = gates.tile([P, NF], F32, tag="sq", name="sq_sb")
                    nc.vector.tensor_tensor(s_acc[bc][:],
                                            s_acc[bc][:].bitcast(F32), nhv,
                                            mybir.AluOpType.add)
                    nc.scalar.activation(sq_sb[:], nhv,
                                         mybir.ActivationFunctionType.Square)
                    nc.vector.tensor_tensor(q_acc[bc][:],
                                            q_acc[bc][:].bitcast(F32),
                                            sq_sb[:],
                                            mybir.AluOpType.add)
                gidx += 1

            nc.gpsimd.dma_start(nhT_d[ht], nh_sb[ht][:])
            if ht + 2 < HT:
                w_cur = w_nxt
                w_nxt = load_wht(ht + 2)
            elif ht + 2 == HT:
                w_cur = w_nxt
            # spread the readout-weight chunks across phase 1
            if ht >= 2:
                nc.gpsimd.dma_start(wout_sb[:, ht - 2, :, :],
                                    wout_d[ht - 2])
        for ot in range(HT - 2, OT):
            nc.gpsimd.dma_start(wout_sb[:, ot, :, :], wout_d[ot])

        # ---- phase 2: LN scale factors + readout --------------------------
        nmu_row = {}
        rstd_row = {}
        rstd_bc = {}

        def emit_stats(bc):
            psum_s = ps.tile([1, NF], F32, tag="c1", name=f"psum_s{bc}")
            nc.tensor.matmul(psum_s[:], ones_col, s_acc[bc][:],
                             start=True, stop=True)
            psum_q = ps.tile([1, NF], F32, tag="d1", name=f"psum_q{bc}")
            nc.tensor.matmul(psum_q[:], ones_col, q_acc[bc][:],
                             start=True, stop=True)

            nmu = rows.tile([1, NF], F32R, tag=f"nmu{bc}", name=f"nmu{bc}")
            nc.vector.tensor_scalar_mul(nmu[:], psum_s[:], -1.0 / H)
            nmu_row[bc] = nmu

            mu2 = rows.tile([1, NF], F32, tag="mu2", name=f"mu2_{bc}")
            nc.vector.tensor_mul(mu2[:], nmu[:].bitcast(F32),
                                 nmu[:].bitcast(F32))
            var = rows.tile([1, NF], F32, tag="var", name=f"var_{bc}")
            nc.vector.tensor_scalar_mul(var[:], psum_q[:], 1.0 / H)
            nc.vector.tensor_tensor(var[:], var[:], mu2[:],
                                    mybir.AluOpType.subtract)
            nc.scalar.activation(var[:], var[:],
                                 mybir.ActivationFunctionType.Sqrt,
                                 bias=eps_sb[:])
            rrow = rows.tile([1, NF], F32R, tag=f"rstd{bc}",
                             name=f"rstd{bc}")
            with nc.allow_low_precision(reason="f32r is fp32-width"):
                nc.vector.reciprocal(rrow[:], var[:])
            rstd_row[bc] = rrow

        def emit_bcast(bc):
            pb = ps.tile([P, NF], F32, tag="c1", name=f"pb{bc}")
            nc.tensor.matmul(pb[:], ones_row, rstd_row[bc][:],
                             start=True, stop=True)
            rb = rows.tile([P, NF], F32, tag=f"rstd_bc{bc}",
                           name=f"rstd_bc{bc}")
            nc.vector.tensor_copy(rb[:], pb[:])
            rstd_bc[bc] = rb

        po_tags = ["a0", "b0", "c0", "d0", "a1", "b1"]
        groups = [(ot, bc) for bc in range(NB) for ot in range(OT)]

        emit_stats(0)
        for i, (ot, bc) in enumerate(groups):
            bs = slice(bc * NF, (bc + 1) * NF)
            po = ps.tile([P, NF], F32, tag=po_tags[i % len(po_tags)],
                         name=f"po_{ot}_{bc}")
            for k in range(HT):
                nc.tensor.matmul(po[:], wout_sb[:, ot, k, :],
                                 nh_sb[k][:, bs],
                                 start=(k == 0), stop=False)
            if i == 0:
                emit_bcast(0)
            elif i == 1:
                emit_stats(1)
            elif i == 2:
                emit_bcast(1)
            # eager finalize: rank-1 -mu*colsum, scale, bias, store
            nc.tensor.matmul(po[:], colsum_sl(ot), nmu_row[bc][:],
                             start=False, stop=True)
            o_sb = gates.tile([P, NF], F32, tag=("t", "u", "sq")[i % 3],
                              name=f"o_{ot}_{bc}")
            nc.vector.tensor_mul(o_sb[:], po[:], rstd_bc[bc][:])
            ob = gates.tile([P, NF], BF16, tag=("r_act", "z_act", "n")[i % 3],
                            name=f"ob_{ot}_{bc}")
            nc.vector.tensor_scalar(ob[:], o_sb[:], bias_col(4, ot), None,
                                    mybir.AluOpType.add)
            nc.sync.dma_start(outT_d[ot][:, bs], ob[:])

    nc.compile()
    return nc


def _swz_w(w):
    # [D, H] -> [HT, P, KT, P] with out[ht, p, t, c] = w[t*P+p, ht*P+c]
    return np.ascontiguousarray(
        w.reshape(KT, P, HT, P).transpose(2, 1, 0, 3)).astype(NPBF16)


def kernel(x, h, Wir, bir, Wiz, biz, Win, bin_, Whr, Whz, Whn, bhn,
           ln_scale, ln_bias, Wout, bout):
    global _COMPILED, LAST_RES
    if _COMPILED is None:
        _COMPILED = _build()
    nc = _COMPILED

    f = lambda a: np.asarray(a, np.float32)
    x, h = f(x), f(h)
    Wout, ln_scale, ln_bias = f(Wout), f(ln_scale), f(ln_bias)

    woutF = ln_scale[:, None] * Wout
    boutF = f(bout) + ln_bias @ Wout
    colsum = ln_scale @ Wout

    wall = np.empty((HT, len(GATES), P, KT, P), dtype=NPBF16)
    for gi, w in enumerate((Wir, Wiz, Win, Whr, Whz, Whn)):
        wall[:, gi] = _swz_w(f(w))
    wout_swz = _swz_w(woutF).reshape(OT, P, KT, P)

    bvec = np.zeros((P, 41), np.float32)
    for vi, v in enumerate((bir, biz, bin_, bhn, boutF)):
        bvec[:, 8 * vi:8 * (vi + 1)] = f(v).reshape(HT, P).T
    bvec[:, 40] = 1.0
    rowv = np.empty((1, P + O), np.float32)
    rowv[0, :P] = 1.0
    rowv[0, P:] = colsum

    common = {"wall": wall, "woutF": wout_swz, "bvec": bvec, "rowv": rowv}
    in_maps = []
    for c in range(NCORES):
        bsl = slice(c * BL, (c + 1) * BL)
        xT = np.ascontiguousarray(x[bsl].T).astype(NPBF16)
        hT = np.ascontiguousarray(h[bsl].T).astype(NPBF16)
        in_maps.append({
            **common,
            "xT": xT.reshape(KT, P, BL),
            "hT": hT.reshape(KT, P, BL),
        })

    res = bass_utils.run_bass_kernel_spmd(nc, in_maps,
                                          core_ids=list(range(NCORES)),
                                          trace=TRACE)
    LAST_RES = res
    nh_parts, out_parts = [], []
    for c in range(NCORES):
        nhT = res.results[c]["nhT"].reshape(H, BL)
        outT = res.results[c]["outT"].reshape(O, BL)
        nh_parts.append(np.asarray(nhT, np.float32).T)
        out_parts.append(np.asarray(outT, np.float32).T)
    new_h = np.ascontiguousarray(np.concatenate(nh_parts, axis=0))
    out = np.ascontiguousarray(np.concatenate(out_parts, axis=0))
    return new_h, out
